# revision 4
# baseline (speedup 1.0000x reference)
"""AFNO2D block-diagonal spectral MLP kernel for 8 Trainium2 NeuronCores.

Math (after simplification of the reference):
  H = W = 128, nb = 8, bs = 96; kept == W so mode truncation is a no-op and
  the imaginary output o2i is discarded by the reference.
  With A1 = w1[0]+w1[1], D1 = w1[0]-w1[1] (same for layer 2):
    o1r = relu(Xk @ (A1/2) + Xn @ (D1/2) + b1[0]/2)
    o1i = relu(Xk @ (D1/2) - Xn @ (D1/2) + b1[1]/2)
    z   = o1r @ (A2/2) + o1i @ (D2/2) + b2[0]/2
    out = x + softshrink(z, 0.01)
  where Xn[b,i,j] = x[b, -i mod H, -j mod W] (pure permutation, done on host
  during sharding). softshrink(z) = relu(z-l) - relu(-z-l)
                                  = relu(z-l) + min(z+l, 0).

Sharding: data-parallel over the 65536 (b,i,j) sites, 8192 per core.

Mirror pairing: sites s and mirror(s) swap (Xk, Xn), so a tile T and its
elementwise-mirror tile T~ share both input tiles, and
Q = Xk@D1h - Xn@D1h satisfies Q(T~) = -Q(T): the o1i matmuls are computed
once per pair. Per 512-site tile that gives 5 matmuls instead of 6 and
halves input DMA. Mirror-fixed sites (i,j in {0,64}) and leftovers go to
two unpaired tiles per core that ship Xn explicitly.

L1 biases ride the matmul via a ones-row (K=97) so the o1r readout for a
whole pair is a single pure-relu [96,1024] op; L2 biases are per-partition
bias APs on the readout ops. All 0.5 scales are folded into the weights.
"""

import numpy as np
import ml_dtypes

import concourse.bass as bass
import concourse.mybir as mybir
from concourse import bacc
from concourse.tile import TileContext
from concourse import bass_utils

BF16 = mybir.dt.bfloat16
F32 = mybir.dt.float32
AF = mybir.ActivationFunctionType
ALU = mybir.AluOpType

B, N, C = 4, 16384, 768
H = W = 128
NB, BS = 8, 96
LAMBDA = 0.01
NCORES = 8
SITES = B * N                      # 65536
SPC = SITES // NCORES              # 8192 sites per core
TILE = 512
NTILES = SPC // TILE               # 16
NPAIRS = 7                         # paired tile-pairs per core (tiles 0..13)
UNP = SPC - NPAIRS * 2 * TILE      # 1024 unpaired sites (tiles 14,15)

_cache = {}


def _build():
    nc = bacc.Bacc("TRN2", target_bir_lowering=False)

    # xk: per-core x values in site_order, channel-major, with ones plane (97)
    xk_d = nc.dram_tensor("xk", [NB, BS + 1, SPC], BF16, kind="ExternalInput")
    # xn for the unpaired tail only
    xn_d = nc.dram_tensor("xn", [NB, BS, UNP], BF16, kind="ExternalInput")
    # weights: w97 kinds (K=97): A1b, D1b ; w96 kinds (K=96): D1h, nD1h, A2h, D2h, nD2h
    w97_d = nc.dram_tensor("w97", [BS + 1, NB * 2 * BS], BF16, kind="ExternalInput")
    w96_d = nc.dram_tensor("w96", [BS, NB * 5 * BS], BF16, kind="ExternalInput")
    # bias APs: kinds: b1i, bias_a (b2/2-l), bias_m (b2/2+l), bias_bm -(b2/2+l)
    bias_d = nc.dram_tensor("b", [BS, NB * 4], F32, kind="ExternalInput")
    out_d = nc.dram_tensor("out", [NB, BS, SPC], BF16, kind="ExternalOutput")

    with TileContext(nc) as tc:
        with (
            tc.tile_pool(name="consts", bufs=1) as consts,
            tc.tile_pool(name="io", bufs=3) as io_pool,
            tc.tile_pool(name="acts", bufs=3) as act_pool,
            tc.tile_pool(name="psum", bufs=3, space="PSUM") as psum_pool,
            tc.tile_pool(name="psq", bufs=2, space="PSUM") as psq_pool,
        ):
            w97 = consts.tile([BS + 1, NB * 2 * BS], BF16)
            nc.sync.dma_start(w97[:], w97_d[:])
            w96 = consts.tile([BS, NB * 5 * BS], BF16)
            nc.sync.dma_start(w96[:], w96_d[:])
            bsb = consts.tile([BS, NB * 4], F32)
            nc.sync.dma_start(bsb[:], bias_d[:])

            def w97AP(n, kind):
                return w97[:, (n * 2 + kind) * BS:(n * 2 + kind + 1) * BS]

            def w96AP(n, kind):
                return w96[:, (n * 5 + kind) * BS:(n * 5 + kind + 1) * BS]

            def bAP(n, kind):
                return bsb[:, n * 4 + kind:n * 4 + kind + 1]

            A1b, D1b = 0, 1
            D1h, nD1h, A2h, D2h, nD2h = 0, 1, 2, 3, 4
            Bi, Ba, Bm, Bbm = 0, 1, 2, 3
            FD = 2 * TILE

            def l2_and_out(n, p2, uv_res, out_t):
                # p2 = z - nothing; readouts add biases. a=relu(z-l), m=min(z+l,0)
                a_t = act_pool.tile([BS, FD], BF16, tag="a")
                nc.scalar.activation(a_t, p2, AF.Relu, bias=bAP(n, Ba), scale=1.0)
                m_t = act_pool.tile([BS, FD], BF16, tag="m")
                if n % 2 == 0:
                    nc.vector.tensor_scalar(m_t, p2, bAP(n, Bm), 0.0, ALU.add, ALU.min)
                else:
                    # b = relu(-z-l); ss = a - b
                    nc.scalar.activation(m_t, p2, AF.Relu, bias=bAP(n, Bbm), scale=-1.0)
                ss_t = act_pool.tile([BS, FD], BF16, tag="ss")
                nc.vector.tensor_tensor(ss_t, a_t, m_t,
                                        ALU.add if n % 2 == 0 else ALU.subtract)
                nc.gpsimd.tensor_tensor(out_t[:, n, :], ss_t, uv_res, ALU.add)

            # ---- paired tiles ----
            for j in range(NPAIRS):
                uv = io_pool.tile([BS + 1, NB, FD], BF16, tag="uv")
                out_t = io_pool.tile([BS, NB, FD], BF16, tag="out")
                nc.sync.dma_start(
                    uv[:], xk_d[:, :, bass.ts(j, FD)].rearrange("n c s -> c n s"))

                for n in range(NB):
                    u97 = uv[:, n, 0:TILE]
                    v97 = uv[:, n, TILE:FD]
                    u96 = uv[0:BS, n, 0:TILE]
                    v96 = uv[0:BS, n, TILE:FD]

                    prpr = psum_pool.tile([BS, FD], F32, tag="big")
                    nc.tensor.matmul(prpr[:, 0:TILE], w97AP(n, A1b), u97,
                                     start=True, stop=False)
                    nc.tensor.matmul(prpr[:, 0:TILE], w96AP(n, D1h), v96,
                                     start=False, stop=True)
                    nc.tensor.matmul(prpr[:, TILE:FD], w97AP(n, A1b), v97,
                                     start=True, stop=False)
                    nc.tensor.matmul(prpr[:, TILE:FD], w96AP(n, D1h), u96,
                                     start=False, stop=True)
                    q = psq_pool.tile([BS, TILE], F32, tag="q")
                    nc.tensor.matmul(q, w96AP(n, D1h), u96, start=True, stop=False)
                    nc.tensor.matmul(q, w96AP(n, nD1h), v96, start=False, stop=True)

                    o1r = act_pool.tile([BS, FD], BF16, tag="o1r")
                    nc.scalar.activation(o1r, prpr, AF.Relu, bias=0.0, scale=1.0)
                    o1i = act_pool.tile([BS, FD], BF16, tag="o1i")
                    # o1i(T) = relu(Q + b1i)
                    nc.vector.tensor_scalar(o1i[:, 0:TILE], q, bAP(n, Bi), 0.0,
                                            ALU.add, ALU.max)
                    # o1i_neg(T~) = min(Q - b1i, 0) = -relu(-Q + b1i)
                    nc.vector.tensor_scalar(o1i[:, TILE:FD], q, bAP(n, Bi), 0.0,
                                            ALU.subtract, ALU.min)

                    p2 = psum_pool.tile([BS, FD], F32, tag="big")
                    nc.tensor.matmul(p2[:, 0:TILE], w96AP(n, A2h), o1r[:, 0:TILE],
                                     start=True, stop=False)
                    nc.tensor.matmul(p2[:, 0:TILE], w96AP(n, D2h), o1i[:, 0:TILE],
                                     start=False, stop=True)
                    nc.tensor.matmul(p2[:, TILE:FD], w96AP(n, A2h), o1r[:, TILE:FD],
                                     start=True, stop=False)
                    nc.tensor.matmul(p2[:, TILE:FD], w96AP(n, nD2h), o1i[:, TILE:FD],
                                     start=False, stop=True)

                    l2_and_out(n, p2, uv[0:BS, n, :], out_t)

                nc.sync.dma_start(
                    out_d[:, :, bass.ts(j, FD)].rearrange("n c s -> c n s"), out_t[:])

            # ---- unpaired tail (2 tiles, FD=1024) ----
            uvU = io_pool.tile([BS + 1, NB, FD], BF16, tag="uv")
            xnU = io_pool.tile([BS, NB, FD], BF16, tag="xnU")
            outU = io_pool.tile([BS, NB, FD], BF16, tag="out")
            nc.sync.dma_start(
                uvU[:], xk_d[:, :, NPAIRS * FD:SPC].rearrange("n c s -> c n s"))
            nc.sync.dma_start(xnU[:], xn_d[:].rearrange("n c s -> c n s"))
            for n in range(NB):
                prpi_r = psum_pool.tile([BS, FD], F32, tag="big")
                prpi_i = psum_pool.tile([BS, FD], F32, tag="big")
                for t in range(2):
                    sl = bass.ts(t, TILE)
                    xk97 = uvU[:, n, sl]
                    xn96 = xnU[:, n, sl]
                    nc.tensor.matmul(prpi_r[:, sl], w97AP(n, A1b), xk97,
                                     start=True, stop=False)
                    nc.tensor.matmul(prpi_r[:, sl], w96AP(n, D1h), xn96,
                                     start=False, stop=True)
                    nc.tensor.matmul(prpi_i[:, sl], w97AP(n, D1b), xk97,
                                     start=True, stop=False)
                    nc.tensor.matmul(prpi_i[:, sl], w96AP(n, nD1h), xn96,
                                     start=False, stop=True)
                o1rU = act_pool.tile([BS, FD], BF16, tag="o1r")
                nc.scalar.activation(o1rU, prpi_r, AF.Relu, bias=0.0, scale=1.0)
                o1iU = act_pool.tile([BS, FD], BF16, tag="o1i")
                nc.vector.tensor_scalar(o1iU, prpi_i, 0.0, None, ALU.max)

                p2U = psum_pool.tile([BS, FD], F32, tag="big")
                for t in range(2):
                    sl = bass.ts(t, TILE)
                    nc.tensor.matmul(p2U[:, sl], w96AP(n, A2h), o1rU[:, sl],
                                     start=True, stop=False)
                    nc.tensor.matmul(p2U[:, sl], w96AP(n, D2h), o1iU[:, sl],
                                     start=False, stop=True)
                l2_and_out(n, p2U, uvU[0:BS, n, :], outU)
            nc.sync.dma_start(
                out_d[:, :, NPAIRS * FD:SPC].rearrange("n c s -> c n s"), outU[:])

    nc.finalize()
    return nc


def _site_order():
    """Global site ordering: per core, 7 mirror-paired tile-pairs then a
    1024-site unpaired tail. Returns (order, n_unpaired_per_core)."""
    b = np.arange(SITES) // N
    ij = np.arange(SITES) % N
    i, jj = ij // W, ij % W
    midx = b * N + ((-i) % H) * W + ((-jj) % W)
    s = np.arange(SITES)
    firsts = s[s < midx]                      # 32760 pair firsts
    fixed = s[s == midx]                      # 16 self-mirrored
    per_core_paired = NPAIRS * TILE           # 3584 pairs per core
    order = np.empty((NCORES, SPC), dtype=np.int64)
    rem = firsts[NCORES * per_core_paired:]   # 4088 leftover pairs
    rem_per_core = len(rem) // NCORES         # 511
    fix_per_core = len(fixed) // NCORES       # 2
    for c in range(NCORES):
        f = firsts[c * per_core_paired:(c + 1) * per_core_paired]
        m = midx[f]
        # tiles 2j (firsts) / 2j+1 (mirrors), aligned elementwise
        paired = np.stack([f.reshape(NPAIRS, TILE), m.reshape(NPAIRS, TILE)],
                          axis=1).reshape(-1)
        r = rem[c * rem_per_core:(c + 1) * rem_per_core]
        fx = fixed[c * fix_per_core:(c + 1) * fix_per_core]
        tail = np.concatenate([r, midx[r], fx])
        order[c] = np.concatenate([paired, tail])
    return order.reshape(-1)


def _host_prep(x, w1, b1, w2, b2):
    bf = ml_dtypes.bfloat16
    order = _cache.setdefault("order", _site_order())
    xf = x.reshape(SITES, C)

    b_ = order // N
    ij = order % N
    i, jj = ij // W, ij % W
    morder = b_ * N + ((-i) % H) * W + ((-jj) % W)

    xk_all = np.empty((NCORES, NB, BS + 1, SPC), dtype=bf)
    xperm = xf[order].T.astype(bf).reshape(NB, BS, NCORES, SPC)
    xk_all[:, :, :BS] = np.moveaxis(xperm, 2, 0)
    xk_all[:, :, BS] = bf(1.0)

    un_idx = np.concatenate([
        morder[c * SPC + NPAIRS * 2 * TILE:(c + 1) * SPC] for c in range(NCORES)])
    xn_un = xf[un_idx].T.astype(bf).reshape(NB, BS, NCORES, UNP)
    xn_all = np.ascontiguousarray(np.moveaxis(xn_un, 2, 0))

    A1h = (w1[0] + w1[1]) * 0.5               # [NB, in, out]
    D1h = (w1[0] - w1[1]) * 0.5
    A2h = (w2[0] + w2[1]) * 0.5
    D2h = (w2[0] - w2[1]) * 0.5
    w97 = np.empty((BS + 1, NB * 2 * BS), dtype=np.float32)
    w96 = np.empty((BS, NB * 5 * BS), dtype=np.float32)
    for n in range(NB):
        for k, (mat, bias) in enumerate(((A1h[n], b1[0, n] * 0.5),
                                         (D1h[n], b1[1, n] * 0.5))):
            w97[:BS, (n * 2 + k) * BS:(n * 2 + k + 1) * BS] = mat
            w97[BS, (n * 2 + k) * BS:(n * 2 + k + 1) * BS] = bias
        for k, mat in enumerate((D1h[n], -D1h[n], A2h[n], D2h[n], -D2h[n])):
            w96[:, (n * 5 + k) * BS:(n * 5 + k + 1) * BS] = mat

    bpack = np.empty((BS, NB * 4), dtype=np.float32)
    for n in range(NB):
        bpack[:, n * 4 + 0] = b1[1, n] * 0.5
        bpack[:, n * 4 + 1] = b2[0, n] * 0.5 - LAMBDA
        bpack[:, n * 4 + 2] = b2[0, n] * 0.5 + LAMBDA
        bpack[:, n * 4 + 3] = -(b2[0, n] * 0.5 + LAMBDA)

    w97b = w97.astype(bf)
    w96b = w96.astype(bf)
    in_maps = []
    for c in range(NCORES):
        in_maps.append({
            "xk": np.ascontiguousarray(xk_all[c]),
            "xn": np.ascontiguousarray(xn_all[c]),
            "w97": w97b,
            "w96": w96b,
            "b": bpack,
        })
    return in_maps


def _assemble(results):
    order = _cache["order"]
    cols = np.concatenate(
        [r["out"].reshape(C, SPC) for r in results], axis=1)   # [C, SITES] in order
    full = np.empty((SITES, C), dtype=np.float32)
    full[order] = cols.T.astype(np.float32)
    return full.reshape(B, N, C)


def _run(x, w1, b1, w2, b2, trace=False):
    if "nc" not in _cache:
        _cache["nc"] = _build()
    nc = _cache["nc"]
    in_maps = _host_prep(x, w1, b1, w2, b2)
    res = bass_utils.run_bass_kernel_spmd(
        nc, in_maps, core_ids=list(range(NCORES)), trace=trace)
    return _assemble(res.results), res


def kernel(x, w1, b1, w2, b2):
    out, _ = _run(x, w1, b1, w2, b2, trace=False)
    return out


# revision 5
# speedup vs baseline: 2.3631x; 2.3631x over previous
"""AFNO2D block-diagonal spectral MLP kernel for 8 Trainium2 NeuronCores.

Math (after simplification of the reference):
  H = W = 128, nb = 8, bs = 96; kept == W so mode truncation is a no-op and
  the imaginary output o2i is discarded by the reference.
  With A1 = w1[0]+w1[1], D1 = w1[0]-w1[1] (same for layer 2):
    o1r = relu(Xk @ (A1/2) + Xn @ (D1/2) + b1[0]/2)
    o1i = relu(Xk @ (D1/2) - Xn @ (D1/2) + b1[1]/2)
    z   = o1r @ (A2/2) + o1i @ (D2/2) + b2[0]/2
    out = x + softshrink(z, 0.01)
  where Xn[b,i,j] = x[b, -i mod H, -j mod W] (pure permutation, done on host
  during sharding). softshrink(z) = relu(z-l) - relu(-z-l)
                                  = relu(z-l) + min(z+l, 0).

Sharding: data-parallel over the 65536 (b,i,j) sites, 8192 per core.

Mirror pairing: sites s and mirror(s) swap (Xk, Xn), so a tile T and its
elementwise-mirror tile T~ share both input tiles, and
Q = Xk@D1h - Xn@D1h satisfies Q(T~) = -Q(T): the o1i matmuls are computed
once per pair. Per 512-site tile that gives 5 matmuls instead of 6 and
halves input DMA. Mirror-fixed sites (i,j in {0,64}) and leftovers go to
two unpaired tiles per core that ship Xn explicitly.

All 0.5 scales fold into the bf16 weights; biases are per-partition bias
APs on the PSUM readouts (o1r readouts for a pair share one bias, so one
merged [96,1024] op serves both tiles).
"""

import numpy as np
import ml_dtypes

import concourse.bass as bass
import concourse.mybir as mybir
from concourse import bacc
from concourse.tile import TileContext
from concourse import bass_utils

BF16 = mybir.dt.bfloat16
F32 = mybir.dt.float32
AF = mybir.ActivationFunctionType
ALU = mybir.AluOpType

B, N, C = 4, 16384, 768
H = W = 128
NB, BS = 8, 96
LAMBDA = 0.01
NCORES = 8
SITES = B * N                      # 65536
SPC = SITES // NCORES              # 8192 sites per core
TILE = 512
FD = 2 * TILE                      # free dim of one group (a tile pair)
NGRP = SPC // FD                   # 8 groups per core
NPAIRS = 7                         # groups 0..6 are mirror pairs
UNP = FD                           # group 7: unpaired tail

_cache = {}


def _build():
    nc = bacc.Bacc("TRN2", target_bir_lowering=False)

    # per-group 2D-contiguous layouts: [group, channel, NB*1024]
    xk_d = nc.dram_tensor("xk", [NGRP, BS, NB * FD], BF16, kind="ExternalInput")
    xn_d = nc.dram_tensor("xn", [BS, NB * FD], BF16, kind="ExternalInput")
    # weight kinds (K=96): A1h, D1h, nD1h, A2h, D2h, nD2h
    w_d = nc.dram_tensor("w", [BS, NB * 6 * BS], BF16, kind="ExternalInput")
    # bias kinds: b1r, b1i, bias_a (b2/2-l), bias_m (b2/2+l), bias_bm -(b2/2+l)
    bias_d = nc.dram_tensor("b", [BS, NB * 5], F32, kind="ExternalInput")
    out_d = nc.dram_tensor("out", [NGRP, BS, NB * FD], BF16, kind="ExternalOutput")

    with TileContext(nc) as tc:
        with (
            tc.tile_pool(name="consts", bufs=1) as consts,
            tc.tile_pool(name="io", bufs=3) as io_pool,
            tc.tile_pool(name="acts", bufs=3) as act_pool,
            tc.tile_pool(name="psum", bufs=3, space="PSUM") as psum_pool,
            tc.tile_pool(name="psq", bufs=2, space="PSUM") as psq_pool,
        ):
            wsb = consts.tile([BS, NB * 6 * BS], BF16)
            nc.sync.dma_start(wsb[:], w_d[:])
            bsb = consts.tile([BS, NB * 5], F32)
            nc.sync.dma_start(bsb[:], bias_d[:])

            def wAP(n, kind):
                return wsb[:, (n * 6 + kind) * BS:(n * 6 + kind + 1) * BS]

            def bAP(n, kind):
                return bsb[:, n * 5 + kind:n * 5 + kind + 1]

            A1h, D1h, nD1h, A2h, D2h, nD2h = range(6)
            Br, Bi, Ba, Bm, Bbm = range(5)

            def l2_and_out(n, p2, res_ap, out_t):
                a_t = act_pool.tile([BS, FD], BF16, tag="a")
                nc.scalar.activation(a_t, p2, AF.Relu, bias=bAP(n, Ba), scale=1.0)
                m_t = act_pool.tile([BS, FD], BF16, tag="m")
                if n % 2 == 0:
                    nc.vector.tensor_scalar(m_t, p2, bAP(n, Bm), 0.0, ALU.add, ALU.min)
                else:
                    nc.scalar.activation(m_t, p2, AF.Relu, bias=bAP(n, Bbm), scale=-1.0)
                ss_t = act_pool.tile([BS, FD], BF16, tag="ss")
                nc.vector.tensor_tensor(ss_t, a_t, m_t,
                                        ALU.add if n % 2 == 0 else ALU.subtract)
                nc.gpsimd.tensor_tensor(out_t[:, n, :], ss_t, res_ap, ALU.add)

            # ---- paired groups ----
            for j in range(NPAIRS):
                uv = io_pool.tile([BS, NB, FD], BF16, tag="uv")
                out_t = io_pool.tile([BS, NB, FD], BF16, tag="out")
                nc.sync.dma_start(uv.rearrange("c n s -> c (n s)"), xk_d[j])

                for n in range(NB):
                    u = uv[:, n, 0:TILE]
                    v = uv[:, n, TILE:FD]

                    prpr = psum_pool.tile([BS, FD], F32, tag="big")
                    nc.tensor.matmul(prpr[:, 0:TILE], wAP(n, A1h), u,
                                     start=True, stop=False)
                    nc.tensor.matmul(prpr[:, 0:TILE], wAP(n, D1h), v,
                                     start=False, stop=True)
                    nc.tensor.matmul(prpr[:, TILE:FD], wAP(n, A1h), v,
                                     start=True, stop=False)
                    nc.tensor.matmul(prpr[:, TILE:FD], wAP(n, D1h), u,
                                     start=False, stop=True)
                    q = psq_pool.tile([BS, TILE], F32, tag="q")
                    nc.tensor.matmul(q, wAP(n, D1h), u, start=True, stop=False)
                    nc.tensor.matmul(q, wAP(n, nD1h), v, start=False, stop=True)

                    o1r = act_pool.tile([BS, FD], BF16, tag="o1r")
                    nc.scalar.activation(o1r, prpr, AF.Relu, bias=bAP(n, Br), scale=1.0)
                    o1i = act_pool.tile([BS, FD], BF16, tag="o1i")
                    # o1i(T) = relu(Q + b1i)
                    nc.vector.tensor_scalar(o1i[:, 0:TILE], q, bAP(n, Bi), 0.0,
                                            ALU.add, ALU.max)
                    # o1i_neg(T~) = min(Q - b1i, 0) = -relu(-Q + b1i)
                    nc.vector.tensor_scalar(o1i[:, TILE:FD], q, bAP(n, Bi), 0.0,
                                            ALU.subtract, ALU.min)

                    p2 = psum_pool.tile([BS, FD], F32, tag="big")
                    nc.tensor.matmul(p2[:, 0:TILE], wAP(n, A2h), o1r[:, 0:TILE],
                                     start=True, stop=False)
                    nc.tensor.matmul(p2[:, 0:TILE], wAP(n, D2h), o1i[:, 0:TILE],
                                     start=False, stop=True)
                    nc.tensor.matmul(p2[:, TILE:FD], wAP(n, A2h), o1r[:, TILE:FD],
                                     start=True, stop=False)
                    nc.tensor.matmul(p2[:, TILE:FD], wAP(n, nD2h), o1i[:, TILE:FD],
                                     start=False, stop=True)

                    l2_and_out(n, p2, uv[:, n, :], out_t)

                nc.sync.dma_start(out_d[j], out_t.rearrange("c n s -> c (n s)"))

            # ---- unpaired tail group ----
            uvU = io_pool.tile([BS, NB, FD], BF16, tag="uv")
            xnU = io_pool.tile([BS, NB, FD], BF16, tag="xnU")
            outU = io_pool.tile([BS, NB, FD], BF16, tag="out")
            nc.sync.dma_start(uvU.rearrange("c n s -> c (n s)"), xk_d[NPAIRS])
            nc.sync.dma_start(xnU.rearrange("c n s -> c (n s)"), xn_d[:])
            for n in range(NB):
                prpi_r = psum_pool.tile([BS, FD], F32, tag="big")
                prpi_i = psum_pool.tile([BS, FD], F32, tag="big")
                for t in range(2):
                    sl = bass.ts(t, TILE)
                    xk_s = uvU[:, n, sl]
                    xn_s = xnU[:, n, sl]
                    nc.tensor.matmul(prpi_r[:, sl], wAP(n, A1h), xk_s,
                                     start=True, stop=False)
                    nc.tensor.matmul(prpi_r[:, sl], wAP(n, D1h), xn_s,
                                     start=False, stop=True)
                    nc.tensor.matmul(prpi_i[:, sl], wAP(n, D1h), xk_s,
                                     start=True, stop=False)
                    nc.tensor.matmul(prpi_i[:, sl], wAP(n, nD1h), xn_s,
                                     start=False, stop=True)
                o1rU = act_pool.tile([BS, FD], BF16, tag="o1r")
                nc.scalar.activation(o1rU, prpi_r, AF.Relu, bias=bAP(n, Br), scale=1.0)
                o1iU = act_pool.tile([BS, FD], BF16, tag="o1i")
                nc.vector.tensor_scalar(o1iU, prpi_i, bAP(n, Bi), 0.0,
                                        ALU.add, ALU.max)

                p2U = psum_pool.tile([BS, FD], F32, tag="big")
                for t in range(2):
                    sl = bass.ts(t, TILE)
                    nc.tensor.matmul(p2U[:, sl], wAP(n, A2h), o1rU[:, sl],
                                     start=True, stop=False)
                    nc.tensor.matmul(p2U[:, sl], wAP(n, D2h), o1iU[:, sl],
                                     start=False, stop=True)
                l2_and_out(n, p2U, uvU[:, n, :], outU)
            nc.sync.dma_start(out_d[NPAIRS], outU.rearrange("c n s -> c (n s)"))

    nc.finalize()
    return nc


def _site_order():
    """Global site ordering: per core, 7 mirror-paired tile-pairs then a
    1024-site unpaired tail."""
    b = np.arange(SITES) // N
    ij = np.arange(SITES) % N
    i, jj = ij // W, ij % W
    midx = b * N + ((-i) % H) * W + ((-jj) % W)
    s = np.arange(SITES)
    firsts = s[s < midx]                      # 32760 pair firsts
    fixed = s[s == midx]                      # 16 self-mirrored
    per_core_paired = NPAIRS * TILE           # 3584 pairs per core
    order = np.empty((NCORES, SPC), dtype=np.int64)
    rem = firsts[NCORES * per_core_paired:]   # 4088 leftover pairs
    rem_per_core = len(rem) // NCORES         # 511
    fix_per_core = len(fixed) // NCORES       # 2
    for c in range(NCORES):
        f = firsts[c * per_core_paired:(c + 1) * per_core_paired]
        m = midx[f]
        paired = np.stack([f.reshape(NPAIRS, TILE), m.reshape(NPAIRS, TILE)],
                          axis=1).reshape(-1)
        r = rem[c * rem_per_core:(c + 1) * rem_per_core]
        fx = fixed[c * fix_per_core:(c + 1) * fix_per_core]
        tail = np.concatenate([r, midx[r], fx])
        order[c] = np.concatenate([paired, tail])
    return order.reshape(-1)


def _host_prep(x, w1, b1, w2, b2):
    bf = ml_dtypes.bfloat16
    order = _cache.setdefault("order", _site_order())
    xf = x.reshape(SITES, C)

    # xk in per-group contiguous layout [core, group, 96, NB*FD]
    xperm = xf[order].T.astype(bf)                     # [C, SITES]
    xk_all = np.ascontiguousarray(
        xperm.reshape(NB, BS, NCORES, NGRP, FD).transpose(2, 3, 1, 0, 4)
    ).reshape(NCORES, NGRP, BS, NB * FD)

    b_ = order // N
    ij = order % N
    i, jj = ij // W, ij % W
    morder = (b_ * N + ((-i) % H) * W + ((-jj) % W)).reshape(NCORES, SPC)
    un_idx = morder[:, NPAIRS * FD:].reshape(-1)
    xn_all = np.ascontiguousarray(
        xf[un_idx].T.astype(bf).reshape(NB, BS, NCORES, UNP).transpose(2, 1, 0, 3)
    ).reshape(NCORES, BS, NB * UNP)

    A1h = (w1[0] + w1[1]) * 0.5               # [NB, in, out]
    D1h = (w1[0] - w1[1]) * 0.5
    A2h = (w2[0] + w2[1]) * 0.5
    D2h = (w2[0] - w2[1]) * 0.5
    wpack = np.empty((BS, NB * 6 * BS), dtype=np.float32)
    for n in range(NB):
        for k, mat in enumerate((A1h[n], D1h[n], -D1h[n], A2h[n], D2h[n], -D2h[n])):
            wpack[:, (n * 6 + k) * BS:(n * 6 + k + 1) * BS] = mat
    wpack = wpack.astype(bf)

    bpack = np.empty((BS, NB * 5), dtype=np.float32)
    for n in range(NB):
        bpack[:, n * 5 + 0] = b1[0, n] * 0.5
        bpack[:, n * 5 + 1] = b1[1, n] * 0.5
        bpack[:, n * 5 + 2] = b2[0, n] * 0.5 - LAMBDA
        bpack[:, n * 5 + 3] = b2[0, n] * 0.5 + LAMBDA
        bpack[:, n * 5 + 4] = -(b2[0, n] * 0.5 + LAMBDA)

    in_maps = []
    for c in range(NCORES):
        in_maps.append({
            "xk": np.ascontiguousarray(xk_all[c]),
            "xn": np.ascontiguousarray(xn_all[c]),
            "w": wpack,
            "b": bpack,
        })
    return in_maps


def _assemble(results):
    order = _cache["order"]
    # out per core: [NGRP, BS, NB*FD] -> [C, SPC] in site order
    cols = np.concatenate(
        [r["out"].reshape(NGRP, BS, NB, FD).transpose(2, 1, 0, 3).reshape(C, SPC)
         for r in results], axis=1)
    full = np.empty((SITES, C), dtype=np.float32)
    full[order] = cols.T.astype(np.float32)
    return full.reshape(B, N, C)


def _run(x, w1, b1, w2, b2, trace=False):
    if "nc" not in _cache:
        _cache["nc"] = _build()
    nc = _cache["nc"]
    in_maps = _host_prep(x, w1, b1, w2, b2)
    res = bass_utils.run_bass_kernel_spmd(
        nc, in_maps, core_ids=list(range(NCORES)), trace=trace)
    return _assemble(res.results), res


def kernel(x, w1, b1, w2, b2):
    out, _ = _run(x, w1, b1, w2, b2, trace=False)
    return out
